# revision 7
# baseline (speedup 1.0000x reference)
"""DIGCN (3-layer directed GCN) Trainium2 Bass kernel, 8-core node-parallel.

Sharding: nodes are range-partitioned across the 8 cores (6250 each). The
dense transform h = x @ W runs on each core for its node slice in transposed
layout (h^T = W^T @ x^T, nodes on the free axis). Slices are AllGathered into
a full row-major h [50000, 128] in every core's HBM; each core then runs the
edge-weighted gather/scatter-add for its destination-node range:

  - per-edge source rows are fetched with the SWDGE dma_gather primitive
    (128 edges land on the 128 SBUF partitions),
  - the weighted segment-sum over destinations is a matmul per 128-edge
    chunk: agg^T[:, tile] += M_c^T @ S_c, where lhsT = gathered messages
    M_c [128e, 128d] and rhs = S_c [128e, 128dst] is a host-built selector
    with S_c[e, j] = w_e * [dst_e == tile_base + j] (PSUM accumulation over
    a tile's chunks).

Edges are grouped on the host by (dst tile of 128 nodes, src < 32768) —
the src split keeps gather indices inside int16 — and padded so every
(tile, half) has a uniform chunk count across tiles AND cores, which keeps
the single SPMD program shape-identical on all 8 cores.

The final FC (emb @ fc_w + fc_b) runs on-device in transposed layout; the
cheap log_softmax over 16 logits runs on the host.
"""

import numpy as np

N = 50000
NC = 8
NPC = N // NC            # 6250 nodes per core
T = (NPC + 127) // 128   # 49 dst tiles per core
D_IN, D_HID, D_OUT = 256, 128, 16
HSPLIT = 32768           # src index split so gather indices fit in int16
CALL_CHUNKS = 32         # 128-edge chunks per dma_gather call (4096 idxs)
SELG = 8                 # selector chunks per DMA group

_PROG_CACHE = {}


def _chunks(total, step):
    out = []
    c0 = 0
    while c0 < total:
        out.append((c0, min(step, total - c0)))
        c0 += step
    return out


def _build_program(C_LO, C_HI):
    import concourse.bacc as bacc
    import concourse.mybir as mybir
    import concourse.tile as tile

    F32 = mybir.dt.float32
    F32R = mybir.dt.float32r  # single-pass PE mode, ~1e-4 matmul accuracy
    I16 = mybir.dt.int16
    AF = mybir.ActivationFunctionType

    TOT = T * (C_LO + C_HI)
    NG = (TOT + SELG - 1) // SELG

    nc = bacc.Bacc("TRN2", target_bir_lowering=False, debug=False, num_devices=NC,
                   num_swdge_queues=4)

    xT_d = nc.dram_tensor("xT", [128, 2, NPC], F32, kind="ExternalInput")
    w1_d = nc.dram_tensor("w1", [128, 2, 128], F32, kind="ExternalInput")
    w2_d = nc.dram_tensor("w2", [128, 128], F32, kind="ExternalInput")
    w3_d = nc.dram_tensor("w3", [128, 128], F32, kind="ExternalInput")
    fcw_d = nc.dram_tensor("fcw", [128, 16], F32, kind="ExternalInput")
    b_d = nc.dram_tensor("b", [128, 3], F32, kind="ExternalInput")
    fcb_d = nc.dram_tensor("fcb", [16, 1], F32, kind="ExternalInput")
    ident_d = nc.dram_tensor("ident", [128, 128], F32, kind="ExternalInput")
    idx_d = nc.dram_tensor("idx", [128, TOT * 8], I16, kind="ExternalInput")
    sel_d = nc.dram_tensor("sel", [NG, 128, SELG * 128], F32R, kind="ExternalInput")

    emb_out = nc.dram_tensor("emb_out", [NPC, 128], F32, kind="ExternalOutput")
    yT_out = nc.dram_tensor("yT_out", [16, NPC], F32, kind="ExternalOutput")

    hsl = [nc.dram_tensor(f"hsl{l}", [NPC, 128], F32R, kind="Internal")
           for l in range(3)]
    hfull = [nc.dram_tensor(f"hfull{l}", [N, 128], F32R, kind="Internal",
                            addr_space="Shared")
             for l in range(3)]

    with tile.TileContext(nc) as tc:
        with (
            tc.tile_pool(name="const", bufs=1) as cpool,
            tc.tile_pool(name="state", bufs=1) as spool,
            tc.tile_pool(name="xin", bufs=2) as xin_pool,
            tc.tile_pool(name="trans", bufs=2) as tpool,
            tc.tile_pool(name="hrow", bufs=4) as hpool,
            tc.tile_pool(name="msg", bufs=3) as mpool,
            tc.tile_pool(name="selp", bufs=2) as selpool,
            tc.tile_pool(name="psA", bufs=2, space="PSUM") as psA,
            tc.tile_pool(name="psT", bufs=2, space="PSUM") as psT,
            tc.tile_pool(name="psG", bufs=4, space="PSUM") as psG,
        ):
            # ---- persistent constants
            w1_sb = cpool.tile([128, 2, 128], F32, tag="w1")
            w2_sb = cpool.tile([128, 128], F32, tag="w2")
            w3_sb = cpool.tile([128, 128], F32, tag="w3")
            fcw_sb = cpool.tile([128, 16], F32, tag="fcw")
            b_sb = cpool.tile([128, 3], F32, tag="b")
            fcb_sb = cpool.tile([16, 1], F32, tag="fcb")
            ident_sb = cpool.tile([128, 128], F32, tag="ident")
            idx_sb = cpool.tile([128, TOT * 8], I16, tag="idx")
            nc.sync.dma_start(w1_sb[:], w1_d[:])
            nc.sync.dma_start(w2_sb[:], w2_d[:])
            nc.sync.dma_start(w3_sb[:], w3_d[:])
            nc.sync.dma_start(fcw_sb[:], fcw_d[:])
            nc.sync.dma_start(b_sb[:], b_d[:])
            nc.sync.dma_start(fcb_sb[:], fcb_d[:])
            nc.sync.dma_start(ident_sb[:], ident_d[:])
            nc.sync.dma_start(idx_sb[:], idx_d[:])

            # ---- persistent state (conv outputs in transposed layout)
            aggT = spool.tile([128, T * 128], F32, tag="aggT")
            xA = spool.tile([128, T * 128], F32, tag="xA")
            xB = spool.tile([128, T * 128], F32, tag="xB")
            xbufs = [None, xA, xB, xA]  # layer l input (l>=1) / output (l+1)

            for l in range(3):
                # ---- dense transform: h_l^T = W_l^T @ x_l^T, then write
                # row-major h_l slice to DRAM via PE transposes.
                KB = 2 if l == 0 else 1
                for (c0, cw) in _chunks(NPC, 512):
                    ps = psA.tile([128, 512], F32, tag="psA")
                    if l == 0:
                        xt = xin_pool.tile([128, 2, 512], F32, tag="xin")
                        nc.sync.dma_start(xt[:, :, :cw], xT_d[:, :, c0:c0 + cw])
                    for kb in range(KB):
                        if l == 0:
                            lhsT = w1_sb[:, kb, :]
                            rhs = xt[:, kb, :cw]
                        else:
                            lhsT = w2_sb[:] if l == 1 else w3_sb[:]
                            rhs = xbufs[l][:, c0:c0 + cw]
                        nc.tensor.matmul(ps[:, :cw], lhsT, rhs,
                                         start=(kb == 0), stop=(kb == KB - 1))
                    th = tpool.tile([128, 512], F32, tag="th")
                    nc.vector.tensor_copy(th[:, :cw], ps[:, :cw])
                    for (s0, sw) in _chunks(cw, 128):
                        ptr = psT.tile([128, 128], F32, tag="psT")
                        nc.tensor.transpose(ptr[:sw, :], th[:, s0:s0 + sw],
                                            ident_sb[:])
                        hr = hpool.tile([128, 128], F32R, tag="hr")
                        nc.vector.tensor_copy(hr[:sw, :], ptr[:sw, :])
                        nc.sync.dma_start(hsl[l][c0 + s0:c0 + s0 + sw, :],
                                          hr[:sw, :])

                # ---- exchange slices -> full row-major h_l on every core
                nc.gpsimd.collective_compute(
                    "AllGather", mybir.AluOpType.bypass,
                    replica_groups=[list(range(NC))],
                    ins=[hsl[l][:]], outs=[hfull[l][:]],
                )

                # ---- message passing for the local dst range
                for phase in range(2):
                    CPT = C_LO if phase == 0 else C_HI
                    tot_ph = T * CPT
                    coff = 0 if phase == 0 else T * C_LO
                    src_ap = hfull[l][:] if phase == 0 else hfull[l][HSPLIT:, :]
                    mt = None
                    sel_t = None
                    pa = None
                    for ct in range(tot_ph):
                        call_slot = ct % CALL_CHUNKS
                        if call_slot == 0:
                            nch = min(CALL_CHUNKS, tot_ph - ct)
                            gc0 = coff + ct
                            mt = mpool.tile([128, CALL_CHUNKS, 128], F32R,
                                            tag="msg")
                            nc.gpsimd.dma_gather(
                                mt[:, :nch, :], src_ap,
                                idx_sb[:, gc0 * 8:(gc0 + nch) * 8],
                                nch * 128, nch * 128, 128,
                                single_packet=False,
                                queue_num=0,
                            )
                        gc = coff + ct
                        g, gi = gc // SELG, gc % SELG
                        if gi == 0 or ct == 0:
                            sel_t = selpool.tile([128, SELG, 128], F32R,
                                                 tag="sel")
                            nc.sync.dma_start(sel_t[:], sel_d[g])
                        ti, cit = ct // CPT, ct % CPT
                        if cit == 0:
                            pa = psG.tile([128, 128], F32, tag="psG")
                        nc.tensor.matmul(pa[:], mt[:, call_slot, :],
                                         sel_t[:, gi, :],
                                         start=(cit == 0),
                                         stop=(cit == CPT - 1))
                        if cit == CPT - 1:
                            dst_sl = aggT[:, ti * 128:(ti + 1) * 128]
                            if phase == 0:
                                nc.vector.tensor_copy(dst_sl, pa[:])
                            else:
                                nc.vector.tensor_add(dst_sl, dst_sl, pa[:])

                # ---- bias (+ relu for layers 0,1) -> next layer input
                xn = xbufs[l + 1]
                func = AF.Relu if l < 2 else AF.Identity
                nc.scalar.activation(xn[:, :NPC], aggT[:, :NPC], func,
                                     bias=b_sb[:, l:l + 1])

            embT = xbufs[3]

            # ---- emb output (row-major) via PE transposes
            for ti in range(T):
                s0 = 128 * ti
                sw = min(128, NPC - s0)
                ptr = psT.tile([128, 128], F32, tag="psT")
                nc.tensor.transpose(ptr[:sw, :], embT[:, s0:s0 + sw],
                                    ident_sb[:])
                hr = hpool.tile([128, 128], F32, tag="hr")
                nc.vector.tensor_copy(hr[:sw, :], ptr[:sw, :])
                nc.sync.dma_start(emb_out[s0:s0 + sw, :], hr[:sw, :])

            # ---- fc: y^T = fc_w^T @ emb^T + fc_b
            for (c0, cw) in _chunks(NPC, 512):
                psy = psA.tile([128, 512], F32, tag="psA")
                nc.tensor.matmul(psy[:16, :cw], fcw_sb[:],
                                 embT[:, c0:c0 + cw], start=True, stop=True)
                yt = tpool.tile([16, 512], F32, tag="yt")
                nc.scalar.activation(yt[:, :cw], psy[:16, :cw], AF.Identity,
                                     bias=fcb_sb[:])
                nc.sync.dma_start(yT_out[:, c0:c0 + cw], yt[:, :cw])

    nc.compile()
    return nc


def _preprocess(x, edge_index, edge_weight):
    """Per-core edge layout + selector construction. Returns the per-core
    input maps (minus the program-independent weight tensors)."""
    src = np.asarray(edge_index[0], dtype=np.int64)
    dst = np.asarray(edge_index[1], dtype=np.int64)
    w = np.asarray(edge_weight, dtype=np.float32)

    core = dst // NPC
    dloc = dst - core * NPC
    tl = dloc >> 7
    half = (src >= HSPLIT).astype(np.int64)

    # group key per edge: (core, tile, half)
    key = (core * T + tl) * 2 + half
    nkeys = NC * T * 2
    counts = np.bincount(key, minlength=nkeys).reshape(NC, T, 2)
    C_LO = int(np.ceil(counts[:, :, 0].max() / 128))
    C_HI = int(np.ceil(counts[:, :, 1].max() / 128))
    TOT = T * (C_LO + C_HI)
    NG = (TOT + SELG - 1) // SELG

    order = np.lexsort((src, key))
    skey = key[order]
    gstart = np.concatenate([[0], np.cumsum(np.bincount(skey, minlength=nkeys))])[:-1]
    rank = np.arange(len(order)) - gstart[skey]

    # base slot of each (core, tile, half) group inside its core's stream
    kk = np.arange(nkeys)
    k_core, rem = kk // (T * 2), kk % (T * 2)
    k_t, k_h = rem // 2, rem % 2
    base = np.where(k_h == 0, k_t * C_LO * 128,
                    (T * C_LO + k_t * C_HI) * 128)
    pos = base[skey] + rank  # position within the owning core's edge stream

    s_src = src[order]
    s_w = w[order]
    s_dloc = (dloc[order] & 127).astype(np.int64)
    s_core = k_core[skey]
    s_half = k_h[skey]

    idx_val = (s_src - s_half * HSPLIT).astype(np.int16)

    per_core = []
    for k in range(NC):
        m = s_core == k
        p = pos[m]
        ia = np.zeros(TOT * 128, np.int16)
        ia[p] = idx_val[m]
        sel = np.zeros((TOT * 128, 128), np.float32)
        sel[p, s_dloc[m]] = s_w[m]
        sel = sel.reshape(TOT, 128, 128)
        if TOT < NG * SELG:
            sel = np.concatenate(
                [sel, np.zeros((NG * SELG - TOT, 128, 128), np.float32)], 0)
        sel = sel.reshape(NG, SELG, 128, 128).transpose(0, 2, 1, 3)
        sel = np.ascontiguousarray(sel.reshape(NG, 128, SELG * 128))
        iw = np.ascontiguousarray(np.tile(ia.reshape(-1, 16).T, (8, 1)))
        per_core.append({"idx": iw, "sel": sel})
    return C_LO, C_HI, per_core


def kernel(**inputs):
    from concourse.bass_utils import run_bass_kernel_spmd

    x = np.asarray(inputs["x"], np.float32)
    edge_index = np.asarray(inputs["edge_index"])
    edge_weight = np.asarray(inputs["edge_weight"], np.float32)
    W1 = np.asarray(inputs["W1"], np.float32)
    b1 = np.asarray(inputs["b1"], np.float32)
    W2 = np.asarray(inputs["W2"], np.float32)
    b2 = np.asarray(inputs["b2"], np.float32)
    W3 = np.asarray(inputs["W3"], np.float32)
    b3 = np.asarray(inputs["b3"], np.float32)
    fc_w = np.asarray(inputs["fc_w"], np.float32)
    fc_b = np.asarray(inputs["fc_b"], np.float32)

    C_LO, C_HI, per_core = _preprocess(x, edge_index, edge_weight)

    key = (C_LO, C_HI)
    if key not in _PROG_CACHE:
        _PROG_CACHE[key] = _build_program(C_LO, C_HI)
    nc = _PROG_CACHE[key]

    w1_h = np.ascontiguousarray(
        W1.reshape(2, 128, 128).transpose(1, 0, 2))  # [128, kb, 128]
    b_h = np.ascontiguousarray(np.stack([b1, b2, b3], 1))  # [128, 3]
    ident = np.eye(128, dtype=np.float32)
    fcb_h = fc_b.reshape(16, 1).astype(np.float32)

    in_maps = []
    for k in range(NC):
        xs = x[k * NPC:(k + 1) * NPC, :]  # [NPC, 256]
        xT = np.ascontiguousarray(
            xs.T.reshape(2, 128, NPC).transpose(1, 0, 2))  # [128, 2, NPC]
        in_maps.append({
            "xT": xT, "w1": w1_h, "w2": W2, "w3": W3, "fcw": fc_w,
            "b": b_h, "fcb": fcb_h, "ident": ident,
            "idx": per_core[k]["idx"], "sel": per_core[k]["sel"],
        })

    res = run_bass_kernel_spmd(nc, in_maps, core_ids=list(range(NC)))

    emb = np.concatenate([res.results[k]["emb_out"] for k in range(NC)], 0)
    y = np.concatenate([res.results[k]["yT_out"] for k in range(NC)], 1).T
    y = y.astype(np.float64)
    m = y.max(axis=1, keepdims=True)
    logp = (y - m) - np.log(np.exp(y - m).sum(axis=1, keepdims=True))
    return emb, logp.astype(np.float32)


# revision 9
# speedup vs baseline: 1.0398x; 1.0398x over previous
"""DIGCN (3-layer directed GCN) Trainium2 Bass kernel, 8-core node-parallel.

Sharding: nodes are range-partitioned across the 8 cores (6250 each). The
dense transform h = x @ W runs on each core for its node slice in transposed
layout (h^T = W^T @ x^T, nodes on the free axis). Slices are AllGathered into
a full row-major h [50000, 128] in every core's HBM; each core then runs the
edge-weighted gather/scatter-add for its destination-node range:

  - per-edge source rows are fetched with the SWDGE dma_gather primitive
    (128 edges land on the 128 SBUF partitions),
  - the weighted segment-sum over destinations is a matmul per 128-edge
    chunk: agg^T[:, tile] += M_c^T @ S_c, where lhsT = gathered messages
    M_c [128e, 128d] and rhs = S_c [128e, 128dst] is a host-built selector
    with S_c[e, j] = w_e * [dst_e == tile_base + j] (PSUM accumulation over
    a tile's chunks).

Edges are grouped on the host by (dst tile of 128 nodes, src < 32768) —
the src split keeps gather indices inside int16 — and padded so every
(tile, half) has a uniform chunk count across tiles AND cores, which keeps
the single SPMD program shape-identical on all 8 cores.

The final FC (emb @ fc_w + fc_b) runs on-device in transposed layout; the
cheap log_softmax over 16 logits runs on the host.
"""

import numpy as np

N = 50000
NC = 8
NPC = N // NC            # 6250 nodes per core
T = (NPC + 127) // 128   # 49 dst tiles per core
D_IN, D_HID, D_OUT = 256, 128, 16
HSPLIT = 32768           # src index split so gather indices fit in int16
CALL_CHUNKS = 32         # 128-edge chunks per dma_gather call (4096 idxs)
SELG = 8                 # selector chunks per DMA group

_PROG_CACHE = {}


def _chunks(total, step):
    out = []
    c0 = 0
    while c0 < total:
        out.append((c0, min(step, total - c0)))
        c0 += step
    return out


def _build_program(C_LO, C_HI):
    import concourse.bacc as bacc
    import concourse.mybir as mybir
    import concourse.tile as tile

    F32 = mybir.dt.float32
    F32R = mybir.dt.float32r  # single-pass PE mode, ~1e-4 matmul accuracy
    I16 = mybir.dt.int16
    AF = mybir.ActivationFunctionType

    TOT = T * (C_LO + C_HI)
    NG = (TOT + SELG - 1) // SELG

    nc = bacc.Bacc("TRN2", target_bir_lowering=False, debug=False, num_devices=NC,
                   num_swdge_queues=4)

    xT_d = nc.dram_tensor("xT", [128, 2, NPC], F32, kind="ExternalInput")
    w1_d = nc.dram_tensor("w1", [128, 2, 128], F32, kind="ExternalInput")
    w2_d = nc.dram_tensor("w2", [128, 128], F32, kind="ExternalInput")
    w3_d = nc.dram_tensor("w3", [128, 128], F32, kind="ExternalInput")
    fcw_d = nc.dram_tensor("fcw", [128, 16], F32, kind="ExternalInput")
    b_d = nc.dram_tensor("b", [128, 3], F32, kind="ExternalInput")
    fcb_d = nc.dram_tensor("fcb", [16, 1], F32, kind="ExternalInput")
    ident_d = nc.dram_tensor("ident", [128, 128], F32, kind="ExternalInput")
    idx_d = nc.dram_tensor("idx", [128, TOT * 8], I16, kind="ExternalInput")
    sel_d = nc.dram_tensor("sel", [NG, 128, SELG * 128], F32R, kind="ExternalInput")

    emb_out = nc.dram_tensor("emb_out", [NPC, 128], F32, kind="ExternalOutput")
    yT_out = nc.dram_tensor("yT_out", [16, NPC], F32, kind="ExternalOutput")

    hsl = [nc.dram_tensor(f"hsl{l}", [NPC, 128], F32R, kind="Internal")
           for l in range(3)]
    hfull = [nc.dram_tensor(f"hfull{l}", [N, 128], F32R, kind="Internal",
                            addr_space="Shared")
             for l in range(3)]

    with tile.TileContext(nc) as tc:
        with (
            tc.tile_pool(name="const", bufs=1) as cpool,
            tc.tile_pool(name="state", bufs=1) as spool,
            tc.tile_pool(name="xin", bufs=2) as xin_pool,
            tc.tile_pool(name="trans", bufs=2) as tpool,
            tc.tile_pool(name="hrow", bufs=4) as hpool,
            tc.tile_pool(name="msg", bufs=3) as mpool,
            tc.tile_pool(name="selp", bufs=2) as selpool,
            tc.tile_pool(name="psA", bufs=2, space="PSUM") as psA,
            tc.tile_pool(name="psT", bufs=2, space="PSUM") as psT,
            tc.tile_pool(name="psG", bufs=4, space="PSUM") as psG,
        ):
            # ---- persistent constants
            w1_sb = cpool.tile([128, 2, 128], F32, tag="w1")
            w2_sb = cpool.tile([128, 128], F32, tag="w2")
            w3_sb = cpool.tile([128, 128], F32, tag="w3")
            fcw_sb = cpool.tile([128, 16], F32, tag="fcw")
            b_sb = cpool.tile([128, 3], F32, tag="b")
            fcb_sb = cpool.tile([16, 1], F32, tag="fcb")
            ident_sb = cpool.tile([128, 128], F32, tag="ident")
            idx_sb = cpool.tile([128, TOT * 8], I16, tag="idx")
            nc.sync.dma_start(w1_sb[:], w1_d[:])
            nc.sync.dma_start(w2_sb[:], w2_d[:])
            nc.sync.dma_start(w3_sb[:], w3_d[:])
            nc.sync.dma_start(fcw_sb[:], fcw_d[:])
            nc.sync.dma_start(b_sb[:], b_d[:])
            nc.sync.dma_start(fcb_sb[:], fcb_d[:])
            nc.sync.dma_start(ident_sb[:], ident_d[:])
            nc.sync.dma_start(idx_sb[:], idx_d[:])

            # ---- persistent state (conv outputs in transposed layout),
            # split into 512-column blocks so Tile's per-tile dependency
            # tracking lets layer l+1's transform start per-block while
            # layer l's hi-phase gathers are still running.
            BLK = [(b * 512, min(512, T * 128 - b * 512))
                   for b in range((T * 128 + 511) // 512)]
            aggTs = [spool.tile([128, w], F32, name=f"aggT{b}", tag=f"aggT{b}")
                     for b, (_, w) in enumerate(BLK)]
            xAs = [spool.tile([128, w], F32, name=f"xA{b}", tag=f"xA{b}")
                   for b, (_, w) in enumerate(BLK)]
            xBs = [spool.tile([128, w], F32, name=f"xB{b}", tag=f"xB{b}")
                   for b, (_, w) in enumerate(BLK)]
            xbufs = [None, xAs, xBs, xAs]

            for l in range(3):
                # ---- dense transform: h_l^T = W_l^T @ x_l^T, then write
                # row-major h_l slice to DRAM via PE transposes.
                KB = 2 if l == 0 else 1
                for bi, (c0, cw) in enumerate(_chunks(NPC, 512)):
                    ps = psA.tile([128, 512], F32, tag="psA")
                    if l == 0:
                        xt = xin_pool.tile([128, 2, 512], F32, tag="xin")
                        nc.sync.dma_start(xt[:, :, :cw], xT_d[:, :, c0:c0 + cw])
                    for kb in range(KB):
                        if l == 0:
                            lhsT = w1_sb[:, kb, :]
                            rhs = xt[:, kb, :cw]
                        else:
                            lhsT = w2_sb[:] if l == 1 else w3_sb[:]
                            rhs = xbufs[l][bi][:, :cw]
                        nc.tensor.matmul(ps[:, :cw], lhsT, rhs,
                                         start=(kb == 0), stop=(kb == KB - 1))
                    th = tpool.tile([128, 512], F32, tag="th")
                    nc.vector.tensor_copy(th[:, :cw], ps[:, :cw])
                    for (s0, sw) in _chunks(cw, 128):
                        ptr = psT.tile([128, 128], F32, tag="psT")
                        nc.tensor.transpose(ptr[:sw, :], th[:, s0:s0 + sw],
                                            ident_sb[:])
                        hr = hpool.tile([128, 128], F32R, tag="hr")
                        nc.vector.tensor_copy(hr[:sw, :], ptr[:sw, :])
                        nc.sync.dma_start(hsl[l][c0 + s0:c0 + s0 + sw, :],
                                          hr[:sw, :])

                # ---- exchange slices -> full row-major h_l on every core
                nc.gpsimd.collective_compute(
                    "AllGather", mybir.AluOpType.bypass,
                    replica_groups=[list(range(NC))],
                    ins=[hsl[l][:]], outs=[hfull[l][:]],
                )

                # ---- message passing for the local dst range
                for phase in range(2):
                    CPT = C_LO if phase == 0 else C_HI
                    tot_ph = T * CPT
                    coff = 0 if phase == 0 else T * C_LO
                    src_ap = hfull[l][:] if phase == 0 else hfull[l][HSPLIT:, :]
                    mt = None
                    sel_t = None
                    pa = None
                    for ct in range(tot_ph):
                        call_slot = ct % CALL_CHUNKS
                        if call_slot == 0:
                            nch = min(CALL_CHUNKS, tot_ph - ct)
                            gc0 = coff + ct
                            mt = mpool.tile([128, CALL_CHUNKS, 128], F32R,
                                            tag="msg")
                            nc.gpsimd.dma_gather(
                                mt[:, :nch, :], src_ap,
                                idx_sb[:, gc0 * 8:(gc0 + nch) * 8],
                                nch * 128, nch * 128, 128,
                                single_packet=False,
                                queue_num=0,
                            )
                        gc = coff + ct
                        g, gi = gc // SELG, gc % SELG
                        if gi == 0 or ct == 0:
                            sel_t = selpool.tile([128, SELG, 128], F32R,
                                                 tag="sel")
                            nc.sync.dma_start(sel_t[:], sel_d[g])
                        ti, cit = ct // CPT, ct % CPT
                        if cit == 0:
                            pa = psG.tile([128, 128], F32, tag="psG")
                        nc.tensor.matmul(pa[:], mt[:, call_slot, :],
                                         sel_t[:, gi, :],
                                         start=(cit == 0),
                                         stop=(cit == CPT - 1))
                        if cit == CPT - 1:
                            col = ti * 128
                            bi, off = col // 512, col % 512
                            dst_sl = aggTs[bi][:, off:off + 128]
                            if phase == 0:
                                nc.vector.tensor_copy(dst_sl, pa[:])
                            else:
                                nc.vector.tensor_add(dst_sl, dst_sl, pa[:])

                # ---- bias (+ relu for layers 0,1) -> next layer input
                func = AF.Relu if l < 2 else AF.Identity
                for b, (_, w) in enumerate(BLK):
                    nc.scalar.activation(xbufs[l + 1][b][:, :w],
                                         aggTs[b][:, :w], func,
                                         bias=b_sb[:, l:l + 1])

            embTs = xbufs[3]

            # ---- emb output (row-major) via PE transposes
            for ti in range(T):
                s0 = 128 * ti
                sw = min(128, NPC - s0)
                bi, off = s0 // 512, s0 % 512
                ptr = psT.tile([128, 128], F32, tag="psT")
                nc.tensor.transpose(ptr[:sw, :], embTs[bi][:, off:off + sw],
                                    ident_sb[:])
                hr = hpool.tile([128, 128], F32, tag="hr")
                nc.vector.tensor_copy(hr[:sw, :], ptr[:sw, :])
                nc.sync.dma_start(emb_out[s0:s0 + sw, :], hr[:sw, :])

            # ---- fc: y^T = fc_w^T @ emb^T + fc_b
            for bi, (c0, cw) in enumerate(_chunks(NPC, 512)):
                psy = psA.tile([128, 512], F32, tag="psA")
                nc.tensor.matmul(psy[:16, :cw], fcw_sb[:],
                                 embTs[bi][:, :cw], start=True, stop=True)
                yt = tpool.tile([16, 512], F32, tag="yt")
                nc.scalar.activation(yt[:, :cw], psy[:16, :cw], AF.Identity,
                                     bias=fcb_sb[:])
                nc.sync.dma_start(yT_out[:, c0:c0 + cw], yt[:, :cw])

    nc.compile()
    return nc


def _preprocess(x, edge_index, edge_weight):
    """Per-core edge layout + selector construction. Returns the per-core
    input maps (minus the program-independent weight tensors)."""
    src = np.asarray(edge_index[0], dtype=np.int64)
    dst = np.asarray(edge_index[1], dtype=np.int64)
    w = np.asarray(edge_weight, dtype=np.float32)

    core = dst // NPC
    dloc = dst - core * NPC
    tl = dloc >> 7
    half = (src >= HSPLIT).astype(np.int64)

    # group key per edge: (core, tile, half)
    key = (core * T + tl) * 2 + half
    nkeys = NC * T * 2
    counts = np.bincount(key, minlength=nkeys).reshape(NC, T, 2)
    C_LO = int(np.ceil(counts[:, :, 0].max() / 128))
    C_HI = int(np.ceil(counts[:, :, 1].max() / 128))
    TOT = T * (C_LO + C_HI)
    NG = (TOT + SELG - 1) // SELG

    order = np.lexsort((src, key))
    skey = key[order]
    gstart = np.concatenate([[0], np.cumsum(np.bincount(skey, minlength=nkeys))])[:-1]
    rank = np.arange(len(order)) - gstart[skey]

    # base slot of each (core, tile, half) group inside its core's stream
    kk = np.arange(nkeys)
    k_core, rem = kk // (T * 2), kk % (T * 2)
    k_t, k_h = rem // 2, rem % 2
    base = np.where(k_h == 0, k_t * C_LO * 128,
                    (T * C_LO + k_t * C_HI) * 128)
    pos = base[skey] + rank  # position within the owning core's edge stream

    s_src = src[order]
    s_w = w[order]
    s_dloc = (dloc[order] & 127).astype(np.int64)
    s_core = k_core[skey]
    s_half = k_h[skey]

    idx_val = (s_src - s_half * HSPLIT).astype(np.int16)

    per_core = []
    for k in range(NC):
        m = s_core == k
        p = pos[m]
        ia = np.zeros(TOT * 128, np.int16)
        ia[p] = idx_val[m]
        sel = np.zeros((TOT * 128, 128), np.float32)
        sel[p, s_dloc[m]] = s_w[m]
        sel = sel.reshape(TOT, 128, 128)
        if TOT < NG * SELG:
            sel = np.concatenate(
                [sel, np.zeros((NG * SELG - TOT, 128, 128), np.float32)], 0)
        sel = sel.reshape(NG, SELG, 128, 128).transpose(0, 2, 1, 3)
        sel = np.ascontiguousarray(sel.reshape(NG, 128, SELG * 128))
        iw = np.ascontiguousarray(np.tile(ia.reshape(-1, 16).T, (8, 1)))
        per_core.append({"idx": iw, "sel": sel})
    return C_LO, C_HI, per_core


def kernel(**inputs):
    from concourse.bass_utils import run_bass_kernel_spmd

    x = np.asarray(inputs["x"], np.float32)
    edge_index = np.asarray(inputs["edge_index"])
    edge_weight = np.asarray(inputs["edge_weight"], np.float32)
    W1 = np.asarray(inputs["W1"], np.float32)
    b1 = np.asarray(inputs["b1"], np.float32)
    W2 = np.asarray(inputs["W2"], np.float32)
    b2 = np.asarray(inputs["b2"], np.float32)
    W3 = np.asarray(inputs["W3"], np.float32)
    b3 = np.asarray(inputs["b3"], np.float32)
    fc_w = np.asarray(inputs["fc_w"], np.float32)
    fc_b = np.asarray(inputs["fc_b"], np.float32)

    C_LO, C_HI, per_core = _preprocess(x, edge_index, edge_weight)

    key = (C_LO, C_HI)
    if key not in _PROG_CACHE:
        _PROG_CACHE[key] = _build_program(C_LO, C_HI)
    nc = _PROG_CACHE[key]

    w1_h = np.ascontiguousarray(
        W1.reshape(2, 128, 128).transpose(1, 0, 2))  # [128, kb, 128]
    b_h = np.ascontiguousarray(np.stack([b1, b2, b3], 1))  # [128, 3]
    ident = np.eye(128, dtype=np.float32)
    fcb_h = fc_b.reshape(16, 1).astype(np.float32)

    in_maps = []
    for k in range(NC):
        xs = x[k * NPC:(k + 1) * NPC, :]  # [NPC, 256]
        xT = np.ascontiguousarray(
            xs.T.reshape(2, 128, NPC).transpose(1, 0, 2))  # [128, 2, NPC]
        in_maps.append({
            "xT": xT, "w1": w1_h, "w2": W2, "w3": W3, "fcw": fc_w,
            "b": b_h, "fcb": fcb_h, "ident": ident,
            "idx": per_core[k]["idx"], "sel": per_core[k]["sel"],
        })

    res = run_bass_kernel_spmd(nc, in_maps, core_ids=list(range(NC)))

    emb = np.concatenate([res.results[k]["emb_out"] for k in range(NC)], 0)
    y = np.concatenate([res.results[k]["yT_out"] for k in range(NC)], 1).T
    y = y.astype(np.float64)
    m = y.max(axis=1, keepdims=True)
    logp = (y - m) - np.log(np.exp(y - m).sum(axis=1, keepdims=True))
    return emb, logp.astype(np.float32)
